# revision 32
# baseline (speedup 1.0000x reference)
"""Trainium2 Bass kernel for CapsuleLayer dynamic routing (8-core SPMD).

Strategy: shard the 2048 input capsules (n) across 8 cores. Each core builds
u_hat = einsum('bni,nio->bno') for its n-slice with W as the PE-stationary
operand so the PSUM output lands with (i4,j32) on partitions -- the native
layout for the routing b-update contraction over j. The o-contraction over n
runs on an n-partition view produced by DMA xbar transposes. Routing's
per-iteration global sum over n is an AllReduce of the tiny [32,32,32] o,
split into two batch-halves so the collective overlaps the o-pass matmuls.
"""
import sys

sys.path.insert(0, "/opt/trn_rl_repo")

import numpy as np
import ml_dtypes

B = 32            # batch
N_TOTAL = 2048    # input capsules
KD = 16           # input capsule dim
NCAP = 32         # output capsules (i)
DIM = 32          # output capsule dim (j)
O = NCAP * DIM    # 1024
NUM_CORES = 8
NL = N_TOTAL // NUM_CORES   # 256 local n
G = NL // 8                 # 32 groups of 8 n
OC = O // 128               # 8 o-chunks
EPS_K = 1e-7
EPS_L2 = 1e-12

_PROG = {}


def _set_dims(ap, dims, offset=None):
    c = ap.copy()
    v = c.ap
    while len(v) > len(dims):
        v.pop()
    while len(v) < len(dims):
        v.insert(0, [0, 1])
    for k, d in enumerate(dims):
        v[k] = list(d)
    if offset is not None:
        c.offset = offset
    return c


def build_program(num_cores):
    import concourse.bass as bass
    import concourse.mybir as mybir
    from concourse import bacc, tile
    from concourse.tile import add_dep_helper

    f32 = mybir.dt.float32
    bf16 = mybir.dt.bfloat16
    AX = mybir.AxisListType
    OP = mybir.AluOpType
    AF = mybir.ActivationFunctionType

    nc = bacc.Bacc(
        "TRN2", target_bir_lowering=False, num_devices=num_cores,
        dynamic_dma_scratch_size=4096,
    )
    rg = [list(range(num_cores))]

    wl_d = nc.dram_tensor("wl", [OC, 128, G * 128], bf16, kind="ExternalInput")
    ubd_d = nc.dram_tensor("ubd", [128, G * 256], bf16, kind="ExternalInput")
    upl_d = nc.dram_tensor("upl", [128, G * B], bf16, kind="ExternalInput")
    out_d = nc.dram_tensor("out", [B, O], f32, kind="ExternalOutput")

    OBP = 8 * 8 * 128     # obd2 col pitch (8192): (b4, oc, 128)
    RING_BUFS = 4         # uht transpose prefetch depth (x 8KB/partition)

    with tile.TileContext(nc) as tc:
        with (
            tc.tile_pool(name="pers", bufs=1) as pers,
            tc.tile_pool(name="dram", bufs=1, space="DRAM") as dram,
        ):
            u_hat = pers.tile([128, B, OC, NL], bf16, tag="u_hat")

            # AllReduce buffers. Round 0 carries o0 in [b, (i,j)] layout;
            # rounds 1,2 carry o_acc in [j, (b,i)] layout. One collective per
            # round: chunked collectives don't overlap each other (single
            # ncfw stream; the gpsimd trigger blocks until the prior op
            # drains), so splitting only adds a second ~10us floor.
            cc_in = [
                dram.tile([B, O], f32, tag=f"cci{t}", name=f"cci{t}")
                for t in range(3)
            ]
            cc_out = [
                dram.tile([B, O], f32, tag=f"cco{t}", name=f"cco{t}")
                for t in range(3)
            ]

            def all_reduce(t):
                if num_cores == 1:
                    nc.gpsimd.dma_start(cc_out[t][:], cc_in[t][:])
                else:
                    nc.gpsimd.collective_compute(
                        "AllReduce", OP.add, replica_groups=rg,
                        ins=[cc_in[t][:].opt()],
                        outs=[cc_out[t][:].opt()],
                    )

            # ---------------- Phase A: load + build u_hat + o0 ----------------
            with (
                tc.tile_pool(name="apool", bufs=1) as apool,
                tc.tile_pool(name="ps_main", bufs=3, space="PSUM") as ps_main,
                tc.tile_pool(name="ps_o0", bufs=1, space="PSUM") as ps_o0,
            ):
                wl_sb = apool.tile([128, OC, G, 128], bf16, tag="wl_sb")
                ubd_sb = apool.tile([128, G, 256], bf16, tag="ubd_sb")
                upl_sb = apool.tile([128, G, B], bf16, tag="upl_sb")
                o0_sb = apool.tile([B, O], f32, tag="o0_sb")
                nc.sync.dma_start(
                    ubd_sb[:].rearrange("p g c -> p (g c)"), ubd_d[:]
                )
                nc.sync.dma_start(
                    upl_sb[:].rearrange("p g c -> p (g c)"), upl_d[:]
                )
                for oc in range(OC):
                    nc.sync.dma_start(
                        wl_sb[:, oc, :, :].rearrange("p g c -> p (g c)"),
                        wl_d[oc, :, :],
                    )

                # o0 = sum_n u_hat FIRST (softmax(0) scale folds into the l2
                # normalize): its AllReduce then hides behind the u_hat
                # matmuls instead of sitting exposed after phase A
                o0a = ps_o0.tile([B, 512], f32, tag="o0a")
                o0b = ps_o0.tile([B, 512], f32, tag="o0b")
                for g in range(G):
                    nc.tensor.matmul(
                        o0a[:], upl_sb[:, g, :],
                        wl_sb[:, 0:4, g, :],
                        start=(g == 0), stop=(g == G - 1),
                        skip_group_check=True,
                    )
                    nc.tensor.matmul(
                        o0b[:], upl_sb[:, g, :],
                        wl_sb[:, 4:8, g, :],
                        start=(g == 0), stop=(g == G - 1),
                        skip_group_check=True,
                    )
                nc.scalar.copy(o0_sb[:, 0:512], o0a[:])
                nc.vector.tensor_copy(o0_sb[:, 512:1024], o0b[:])
                nc.scalar.dma_start(cc_in[0][:], o0_sb[:])
                all_reduce(0)

                for oc in range(OC):
                    for g4 in range(G // 4):
                        ps = ps_main.tile([128, 1024], f32, tag="ps")
                        for gh in range(4):
                            g = g4 * 4 + gh
                            nc.tensor.matmul(
                                ps[:, gh * 256:(gh + 1) * 256],
                                wl_sb[:, oc, g, :], ubd_sb[:, g, :],
                                start=True, stop=True,
                                skip_group_check=True,
                            )
                        # psum cols (gh, b, n8) -> u_hat[:, b, oc, g4*32+gh*8+n8]
                        g = g4 * 4
                        dst = _set_dims(
                            u_hat[:, :, oc, 0],
                            [[B * OC * NL, 128], [8, 4], [OC * NL, B], [1, 8]],
                            offset=oc * NL + g * 8,
                        )
                        src = ps[:].rearrange("p (h b n) -> p h b n", h=4, b=B)
                        if g4 % 2 == 0:
                            nc.scalar.copy(dst, src)
                        else:
                            nc.vector.tensor_copy(dst, src)

            # ---------------- routing iterations ----------------
            with (
                tc.tile_pool(name="rout", bufs=1) as rout,
                tc.tile_pool(name="ring", bufs=RING_BUFS) as ring,
                tc.tile_pool(name="bu", bufs=2) as bu,
                tc.tile_pool(name="tring", bufs=2) as tring,
                tc.tile_pool(name="ps_b", bufs=2, space="PSUM") as ps_b,
                tc.tile_pool(name="ps_o", bufs=2, space="PSUM") as ps_o,
            ):
                tr_scr = rout.tile([128, 1024], f32, tag="tr_scr")
                onrm = rout.tile([B, O], f32, tag="onrm")
                onl = rout.tile([B, O], f32, tag="onl")
                # aliases (disjoint lifetimes within an iteration):
                # o_t lives in rows 0-31 of tr_scr between the normalize
                # v.transpose and the obd2 scatter; o_acc overlays onl after
                # the normalize output has been consumed by that scatter.
                o_t = tr_scr
                s2 = rout.tile([B, NCAP], f32, tag="s2")
                s2b = rout.tile([B, NCAP], f32, tag="s2b")
                s2c = rout.tile([B, NCAP], f32, tag="s2c")
                rinv = rout.tile([B, NCAP], f32, tag="rinv")
                obd2 = rout.tile([128, 8, 8, 128], bf16, tag="obd2")
                blog = rout.tile([128, B, 2, NCAP], f32, tag="blog")
                c_sb = rout.tile([128, B, 2, NCAP], bf16, tag="c_sb")
                mx = rout.tile([128, B, 2], f32, tag="mx")
                sm = rout.tile([128, B, 2], f32, tag="sm")
                smr = rout.tile([128, B, 2], f32, tag="smr")

                ms2a = nc.gpsimd.memset(obd2[:, 0:4, :, :], 0.0)
                ms2b = nc.vector.memset(obd2[:, 4:8, :, :], 0.0)

                for t in range(3):
                    final = t == 2
                    ji = t > 0  # cc[0] is [b,(i,j)]; cc[1:] are [j,(b,i)]
                    # ---- load global o. Round 0 arrives as [b, (i,j)];
                    # rounds 1,2 as [i, (b,j)] (pre-transposed on chip before
                    # shipping). Both are plain contiguous loads, and the
                    # whole normalize chain below is layout-agnostic: both
                    # views are [32, 32-outer, 32-reduce] with identical APs.
                    nc.scalar.dma_start(onrm[:], cc_out[t][:])
                    nc.scalar.square(onl[:], onrm[:])
                    nc.vector.tensor_reduce(
                        s2[:], onl[:].rearrange("b (i j) -> b i j", i=NCAP),
                        axis=AX.X, op=OP.add,
                    )
                    if not final:
                        nc.vector.tensor_scalar_max(s2b[:], s2[:], EPS_L2)
                        nc.scalar.sqrt(s2c[:], s2b[:])
                        nc.vector.reciprocal(rinv[:], s2c[:])
                    else:
                        # squash scale = s2 / ((1+s2) * sqrt(s2+eps))
                        nc.vector.tensor_scalar_add(s2b[:], s2[:], EPS_K)
                        nc.scalar.sqrt(s2b[:], s2b[:])
                        nc.vector.tensor_scalar_add(s2c[:], s2[:], 1.0)
                        nc.vector.tensor_mul(s2c[:], s2c[:], s2b[:])
                        nc.vector.reciprocal(s2b[:], s2c[:])
                        nc.vector.tensor_mul(rinv[:], s2b[:], s2[:])
                    sc_b = _set_dims(rinv[:], [[NCAP, B], [1, NCAP], [0, DIM]])
                    nc.vector.tensor_mul(
                        onl[:].rearrange("b (i j) -> b i j", i=NCAP),
                        onrm[:].rearrange("b (i j) -> b i j", i=NCAP),
                        sc_b,
                    )
                    if final:
                        # onl is [i, (b,j)] -> out[b, i*32+j]
                        dstF = _set_dims(
                            out_d[:], [[32, 32], [1024, 32], [1, 32]]
                        )
                        nc.sync.dma_start(dstF, onl[:])
                        break

                    # ---- scatter normalized o into block-diag obd2 (bf16)
                    # 32x32 block transpose into o_t:
                    #   round 0: onl[b, (i,j)] -> o_t[j, (i,b)]
                    #   rounds 1+: onl[i, (b,j)] -> o_t[j, (b,i)]
                    nc.vector.transpose(o_t[0:B, :], onl[:])
                    # obd2[32m+j, b4*1024+oc*132+bq*32+m] = o(b4*4+bq, oc*4+m, j)
                    for m in range(4):
                        if not ji:
                            src_e = _set_dims(
                                o_t[:],
                                [[1024, DIM], [4, 8], [128, 8], [1, 4]],
                                offset=DIM * m,
                            )
                        else:
                            src_e = _set_dims(
                                o_t[:],
                                [[1024, DIM], [128, 8], [4, 8], [32, 4]],
                                offset=m,
                            )
                        dst_e = _set_dims(
                            obd2[:],
                            [[OBP, DIM], [1024, 8], [132, 8], [32, 4]],
                            offset=(32 * m) * OBP + m,
                        )
                        if m % 2 == 0:
                            ec = nc.vector.tensor_copy(dst_e, src_e)
                        else:
                            ec = nc.scalar.copy(dst_e, src_e)
                        if t == 0:
                            for ms in (ms2a, ms2b):
                                add_dep_helper(
                                    ec.ins, ms.ins, sync=True,
                                    reason="mask copy after memset",
                                )

                    # ---- b-update: per b4 (4 batches), 8 oc x 2 half matmuls
                    # (512-col halves keep each matmul within one PSUM bank)
                    for b4 in range(8):
                        psb = ps_b.tile([128, 1024], f32, tag="psb")
                        for oc in range(OC):
                            for bh in range(2):
                                nc.tensor.matmul(
                                    psb[:, bh * 512:(bh + 1) * 512],
                                    obd2[:, b4, oc, :],
                                    u_hat[:, 4 * b4 + 2 * bh:4 * b4 + 2 * bh + 2,
                                          oc, :],
                                    start=(oc == 0), stop=(oc == OC - 1),
                                    skip_group_check=True,
                                )
                        # bf16 logits: |b| ~ O(1), bf16 rel err ~0.4% keeps the
                        # routing softmax well inside the 2e-2 tolerance
                        sthi = bu.tile([128, 1024], bf16, tag="sthi")
                        nc.scalar.copy(sthi[:], psb[:])
                        thi = tring.tile([128, 8, 128], bf16, tag="thi")
                        nc.sync.dma_start_transpose(thi[:], sthi[:])
                        # pick diag: blog[p, 4*b4+bq, nh, i] = t[p, bq*2+nh, bq*32+i]
                        thv = _set_dims(
                            thi[:], [[1024, 128], [288, 4], [128, 2], [1, 32]]
                        )
                        nc.vector.tensor_copy(
                            blog[:, 4 * b4:4 * (b4 + 1), :, :], thv
                        )

                        # ---- softmax over i, per 8-batch group
                        if b4 % 2 == 1:
                            g8 = b4 // 2
                            bl = blog[:, 8 * g8:8 * (g8 + 1), :, :]
                            bl3 = bl.rearrange("p b h i -> p (b h) i")
                            mxs = mx[:, 8 * g8:8 * (g8 + 1), :]
                            sms = sm[:, 8 * g8:8 * (g8 + 1), :]
                            smrs = smr[:, 8 * g8:8 * (g8 + 1), :]
                            nc.vector.tensor_reduce(
                                mxs, bl, axis=AX.X, op=OP.max
                            )
                            mxb = _set_dims(
                                mx[:, 0, 0],
                                [[2 * B, 128], [1, 16], [0, NCAP]],
                                offset=16 * g8,
                            )
                            nc.vector.tensor_sub(bl3, bl3, mxb)
                            nc.scalar.activation(bl, bl, AF.Exp)
                            nc.vector.tensor_reduce(
                                sms, bl, axis=AX.X, op=OP.add
                            )
                            nc.vector.reciprocal(smrs, sms)
                            smb = _set_dims(
                                smr[:, 0, 0],
                                [[2 * B, 128], [1, 16], [0, NCAP]],
                                offset=16 * g8,
                            )
                            nc.vector.tensor_mul(
                                c_sb[:, 8 * g8:8 * (g8 + 1), :, :].rearrange(
                                    "p b h i -> p (b h) i"
                                ),
                                bl3, smb,
                            )

                    # ---- o-pass: xbar-transpose u_hat per 2b, matmul with c
                    # issue the first ring-depth transposes early so they run
                    # during the b-update (the data, u_hat, is ready all along)
                    uhts = []
                    for b in range(0, 2 * RING_BUFS, 2):
                        uht = ring.tile([128, 32, 128], bf16, tag="uht")
                        nc.sync.dma_start_transpose(
                            uht[:],
                            u_hat[:, b:b + 2, :, :].rearrange(
                                "p b a n -> p (b a n)"
                            ),
                        )
                        uhts.append(uht)
                    for b in range(B):
                        cg = b & 3
                        if cg == 0:
                            pso = ps_o.tile([128, 1024], f32, tag="pso")
                        if b % 2 == 0:
                            if b < 2 * RING_BUFS:
                                uht = uhts[b // 2]
                            else:
                                uht = ring.tile([128, 32, 128], bf16, tag="uht")
                                nc.sync.dma_start_transpose(
                                    uht[:],
                                    u_hat[:, b:b + 2, :, :].rearrange(
                                        "p b a n -> p (b a n)"
                                    ),
                                )
                        b1 = b & 1
                        for nh in range(2):
                            lhs = c_sb[:, b, nh, :]
                            for oh in range(2):
                                rhs = _set_dims(
                                    uht[:],
                                    [[32 * 128, 128], [256, 4], [1, 128]],
                                    offset=(16 * b1 + 8 * oh + nh) * 128,
                                )
                                nc.tensor.matmul(
                                    pso[32 * cg:32 * cg + 32,
                                        oh * 512:(oh + 1) * 512],
                                    lhs, rhs,
                                    start=(nh == 0), stop=(nh == 1),
                                    tile_position=(0, 32 * cg),
                                    skip_group_check=True,
                                )
                        if cg == 3:
                            # 32x32 block transpose; diag becomes stride-33 cols
                            nc.vector.transpose(tr_scr[:], pso[:])
                            for c2 in range(4):
                                bb = b - 3 + c2
                                diag = _set_dims(
                                    tr_scr[:], [[1024, 32], [33, DIM]],
                                    offset=(32 * c2) * 1024,
                                )
                                # o_acc[j, bb, i] overlays onl
                                dst_a = _set_dims(
                                    onl[:], [[O, DIM], [1, NCAP]],
                                    offset=bb * NCAP,
                                )
                                if c2 % 2 == 0:
                                    nc.scalar.copy(dst_a, diag)
                                else:
                                    nc.vector.tensor_copy(dst_a, diag)
                    # o_acc [j, (b,i)] (overlaid on onl) -> transpose on chip
                    # to [i, (b,j)] so the AllReduce buffers stay contiguous
                    # for both the ship and the next round's load
                    nc.vector.transpose(o_t[0:B, :], onl[:])
                    nc.scalar.dma_start(cc_in[t + 1][:], o_t[0:B, :])
                    all_reduce(t + 1)

    nc.compile()
    return nc


def host_prep(u_vecs, W, core):
    ns = slice(core * NL, (core + 1) * NL)
    Wc = np.asarray(W[ns], dtype=np.float32)             # [NL, 16, 1024]
    uc = np.asarray(u_vecs[:, ns, :], dtype=np.float32)  # [B, NL, 16]
    bf = ml_dtypes.bfloat16

    # wl[oc, n8*16+k, g*128+c] = W[g*8+n8, k, oc*128+c]
    wl = (
        Wc.reshape(G, 8, KD, OC, 128)
        .transpose(3, 1, 2, 0, 4)
        .reshape(OC, 128, G * 128)
        .astype(bf)
    )
    tmp = uc.transpose(1, 2, 0).reshape(G, 8, KD, B)     # [g, n8, k, b]
    # ubd[n8*16+k, g*256 + b*8+n8'] = u[b, g*8+n8, k] * (n8 == n8')
    ubd = np.zeros((8, KD, G, B, 8), dtype=np.float32)
    for n8 in range(8):
        ubd[n8, :, :, :, n8] = tmp[:, n8].transpose(1, 0, 2)
    ubd = ubd.reshape(128, G * B * 8).astype(bf)
    # upl[n8*16+k, g*32+b] = u[b, g*8+n8, k]
    upl = tmp.transpose(1, 2, 0, 3).reshape(128, G * B).astype(bf)
    return {"wl": wl, "ubd": ubd, "upl": upl}


def kernel(u_vecs, W):
    from concourse import bass_utils

    if "prog" not in _PROG:
        _PROG["prog"] = build_program(NUM_CORES)
    nc = _PROG["prog"]
    in_maps = [host_prep(u_vecs, W, c) for c in range(NUM_CORES)]
    res = bass_utils.run_bass_kernel_spmd(
        nc, in_maps, core_ids=list(range(NUM_CORES))
    )
    out = np.asarray(res.results[0]["out"], dtype=np.float32)
    return out.reshape(B, NCAP, DIM)


# revision 34
# speedup vs baseline: 1.1044x; 1.1044x over previous
"""Trainium2 Bass kernel for CapsuleLayer dynamic routing (8-core SPMD).

Strategy: shard the 2048 input capsules (n) across 8 cores. Each core builds
u_hat = einsum('bni,nio->bno') for its n-slice with W as the PE-stationary
operand so the PSUM output lands with (i4,j32) on partitions -- the native
layout for the routing b-update contraction over j. The o-contraction over n
runs on an n-partition view produced by DMA xbar transposes. Routing's
per-iteration global sum over n is an AllReduce of the tiny [32,32,32] o,
split into two batch-halves so the collective overlaps the o-pass matmuls.
"""
import sys

sys.path.insert(0, "/opt/trn_rl_repo")

import numpy as np
import ml_dtypes

B = 32            # batch
N_TOTAL = 2048    # input capsules
KD = 16           # input capsule dim
NCAP = 32         # output capsules (i)
DIM = 32          # output capsule dim (j)
O = NCAP * DIM    # 1024
NUM_CORES = 8
NL = N_TOTAL // NUM_CORES   # 256 local n
G = NL // 8                 # 32 groups of 8 n
OC = O // 128               # 8 o-chunks
EPS_K = 1e-7
EPS_L2 = 1e-12

_PROG = {}


def _set_dims(ap, dims, offset=None):
    c = ap.copy()
    v = c.ap
    while len(v) > len(dims):
        v.pop()
    while len(v) < len(dims):
        v.insert(0, [0, 1])
    for k, d in enumerate(dims):
        v[k] = list(d)
    if offset is not None:
        c.offset = offset
    return c


def build_program(num_cores):
    import concourse.bass as bass
    import concourse.mybir as mybir
    from concourse import bacc, tile
    from concourse.tile import add_dep_helper

    f32 = mybir.dt.float32
    bf16 = mybir.dt.bfloat16
    AX = mybir.AxisListType
    OP = mybir.AluOpType
    AF = mybir.ActivationFunctionType

    nc = bacc.Bacc(
        "TRN2", target_bir_lowering=False, num_devices=num_cores,
        dynamic_dma_scratch_size=4096,
    )
    rg = [list(range(num_cores))]

    wl_d = nc.dram_tensor("wl", [OC, 128, G * 128], bf16, kind="ExternalInput")
    ubd_d = nc.dram_tensor("ubd", [128, G * 256], bf16, kind="ExternalInput")
    upl_d = nc.dram_tensor("upl", [128, G * B], bf16, kind="ExternalInput")
    out_d = nc.dram_tensor("out", [B, O], f32, kind="ExternalOutput")

    OBP = 8 * 8 * 128     # obd2 col pitch (8192): (b4, oc, 128)
    RING_BUFS = 5         # uht transpose prefetch depth (x 8KB/partition)

    with tile.TileContext(nc) as tc:
        with (
            tc.tile_pool(name="pers", bufs=1) as pers,
            tc.tile_pool(name="dram", bufs=1, space="DRAM") as dram,
        ):
            u_hat = pers.tile([128, B, OC, NL], bf16, tag="u_hat")

            # AllReduce buffers. Round 0 carries o0 in [b, (i,j)] layout;
            # rounds 1,2 carry o_acc in [j, (b,i)] layout. One collective per
            # round: chunked collectives don't overlap each other (single
            # ncfw stream; the gpsimd trigger blocks until the prior op
            # drains), so splitting only adds a second ~10us floor.
            cc_in = [
                dram.tile([B, O], f32, tag=f"cci{t}", name=f"cci{t}")
                for t in range(3)
            ]
            cc_out = [
                dram.tile([B, O], f32, tag=f"cco{t}", name=f"cco{t}")
                for t in range(3)
            ]

            def all_reduce(t):
                if num_cores == 1:
                    nc.gpsimd.dma_start(cc_out[t][:], cc_in[t][:])
                else:
                    nc.gpsimd.collective_compute(
                        "AllReduce", OP.add, replica_groups=rg,
                        ins=[cc_in[t][:].opt()],
                        outs=[cc_out[t][:].opt()],
                    )

            # ---------------- Phase A: load + build u_hat + o0 ----------------
            with (
                tc.tile_pool(name="apool", bufs=1) as apool,
                tc.tile_pool(name="ps_main", bufs=3, space="PSUM") as ps_main,
                tc.tile_pool(name="ps_o0", bufs=1, space="PSUM") as ps_o0,
            ):
                wl_sb = apool.tile([128, OC, G, 128], bf16, tag="wl_sb")
                ubd_sb = apool.tile([128, G, 256], bf16, tag="ubd_sb")
                upl_sb = apool.tile([128, G, B], bf16, tag="upl_sb")
                o0_sb = apool.tile([B, O], f32, tag="o0_sb")
                nc.sync.dma_start(
                    ubd_sb[:].rearrange("p g c -> p (g c)"), ubd_d[:]
                )
                nc.sync.dma_start(
                    upl_sb[:].rearrange("p g c -> p (g c)"), upl_d[:]
                )
                for oc in range(OC):
                    nc.sync.dma_start(
                        wl_sb[:, oc, :, :].rearrange("p g c -> p (g c)"),
                        wl_d[oc, :, :],
                    )

                # o0 = sum_n u_hat FIRST (softmax(0) scale folds into the l2
                # normalize): its AllReduce then hides behind the u_hat
                # matmuls instead of sitting exposed after phase A
                o0a = ps_o0.tile([B, 512], f32, tag="o0a")
                o0b = ps_o0.tile([B, 512], f32, tag="o0b")
                for g in range(G):
                    nc.tensor.matmul(
                        o0a[:], upl_sb[:, g, :],
                        wl_sb[:, 0:4, g, :],
                        start=(g == 0), stop=(g == G - 1),
                        skip_group_check=True,
                    )
                    nc.tensor.matmul(
                        o0b[:], upl_sb[:, g, :],
                        wl_sb[:, 4:8, g, :],
                        start=(g == 0), stop=(g == G - 1),
                        skip_group_check=True,
                    )
                nc.scalar.copy(o0_sb[:, 0:512], o0a[:])
                nc.vector.tensor_copy(o0_sb[:, 512:1024], o0b[:])
                nc.scalar.dma_start(cc_in[0][:], o0_sb[:])
                all_reduce(0)

                for oc in range(OC):
                    for g4 in range(G // 4):
                        ps = ps_main.tile([128, 1024], f32, tag="ps")
                        for gh in range(4):
                            g = g4 * 4 + gh
                            nc.tensor.matmul(
                                ps[:, gh * 256:(gh + 1) * 256],
                                wl_sb[:, oc, g, :], ubd_sb[:, g, :],
                                start=True, stop=True,
                                skip_group_check=True,
                            )
                        # psum cols (gh, b, n8) -> u_hat[:, b, oc, g4*32+gh*8+n8]
                        g = g4 * 4
                        dst = _set_dims(
                            u_hat[:, :, oc, 0],
                            [[B * OC * NL, 128], [8, 4], [OC * NL, B], [1, 8]],
                            offset=oc * NL + g * 8,
                        )
                        src = ps[:].rearrange("p (h b n) -> p h b n", h=4, b=B)
                        if g4 % 2 == 0:
                            nc.scalar.copy(dst, src)
                        else:
                            nc.vector.tensor_copy(dst, src)

            # ---------------- routing iterations ----------------
            with (
                tc.tile_pool(name="rout", bufs=1) as rout,
                tc.tile_pool(name="ring", bufs=RING_BUFS) as ring,
                tc.tile_pool(name="bu", bufs=2) as bu,
                tc.tile_pool(name="tring", bufs=2) as tring,
                tc.tile_pool(name="ps_b", bufs=2, space="PSUM") as ps_b,
                tc.tile_pool(name="ps_o", bufs=2, space="PSUM") as ps_o,
            ):
                tr_scr = rout.tile([128, 1024], f32, tag="tr_scr")
                onrm = rout.tile([B, O], f32, tag="onrm")
                onl = rout.tile([B, O], f32, tag="onl")
                # aliases (disjoint lifetimes within an iteration):
                # o_t lives in rows 0-31 of tr_scr between the normalize
                # v.transpose and the obd2 scatter; o_acc overlays onl after
                # the normalize output has been consumed by that scatter.
                o_t = tr_scr
                s2 = rout.tile([B, NCAP], f32, tag="s2")
                s2b = rout.tile([B, NCAP], f32, tag="s2b")
                s2c = rout.tile([B, NCAP], f32, tag="s2c")
                rinv = rout.tile([B, NCAP], f32, tag="rinv")
                obd2 = rout.tile([128, 8, 8, 128], bf16, tag="obd2")
                blog = rout.tile([128, B, 2, NCAP], f32, tag="blog")
                c_sb = rout.tile([128, B, 2, NCAP], bf16, tag="c_sb")
                mx = rout.tile([128, B, 2], f32, tag="mx")
                sm = rout.tile([128, B, 2], f32, tag="sm")
                smr = rout.tile([128, B, 2], f32, tag="smr")

                ms2a = nc.gpsimd.memset(obd2[:, 0:4, :, :], 0.0)
                ms2b = nc.vector.memset(obd2[:, 4:8, :, :], 0.0)

                for t in range(3):
                    final = t == 2
                    ji = t > 0  # cc[0] is [b,(i,j)]; cc[1:] are [j,(b,i)]
                    # ---- load global o. Round 0 arrives as [b, (i,j)];
                    # rounds 1,2 as [i, (b,j)] (pre-transposed on chip before
                    # shipping). Both are plain contiguous loads, and the
                    # whole normalize chain below is layout-agnostic: both
                    # views are [32, 32-outer, 32-reduce] with identical APs.
                    nc.scalar.dma_start(onrm[:], cc_out[t][:])
                    nc.scalar.square(onl[:], onrm[:])
                    nc.vector.tensor_reduce(
                        s2[:], onl[:].rearrange("b (i j) -> b i j", i=NCAP),
                        axis=AX.X, op=OP.add,
                    )
                    if not final:
                        nc.vector.tensor_scalar_max(s2b[:], s2[:], EPS_L2)
                        nc.scalar.sqrt(s2c[:], s2b[:])
                        nc.vector.reciprocal(rinv[:], s2c[:])
                    else:
                        # squash scale = s2 / ((1+s2) * sqrt(s2+eps))
                        nc.vector.tensor_scalar_add(s2b[:], s2[:], EPS_K)
                        nc.scalar.sqrt(s2b[:], s2b[:])
                        nc.vector.tensor_scalar_add(s2c[:], s2[:], 1.0)
                        nc.vector.tensor_mul(s2c[:], s2c[:], s2b[:])
                        nc.vector.reciprocal(s2b[:], s2c[:])
                        nc.vector.tensor_mul(rinv[:], s2b[:], s2[:])
                    sc_b = _set_dims(rinv[:], [[NCAP, B], [1, NCAP], [0, DIM]])
                    nc.vector.tensor_mul(
                        onl[:].rearrange("b (i j) -> b i j", i=NCAP),
                        onrm[:].rearrange("b (i j) -> b i j", i=NCAP),
                        sc_b,
                    )
                    if final:
                        # onl is [i, (b,j)] -> out[b, i*32+j]
                        dstF = _set_dims(
                            out_d[:], [[32, 32], [1024, 32], [1, 32]]
                        )
                        nc.sync.dma_start(dstF, onl[:])
                        break

                    # ---- scatter normalized o into block-diag obd2 (bf16)
                    # 32x32 block transpose into o_t:
                    #   round 0: onl[b, (i,j)] -> o_t[j, (i,b)]
                    #   rounds 1+: onl[i, (b,j)] -> o_t[j, (b,i)]
                    nc.vector.transpose(o_t[0:B, :], onl[:])
                    # obd2[32m+j, b4*1024+oc*132+bq*32+m] = o(b4*4+bq, oc*4+m, j)
                    for m in range(4):
                        if not ji:
                            src_e = _set_dims(
                                o_t[:],
                                [[1024, DIM], [4, 8], [128, 8], [1, 4]],
                                offset=DIM * m,
                            )
                        else:
                            src_e = _set_dims(
                                o_t[:],
                                [[1024, DIM], [128, 8], [4, 8], [32, 4]],
                                offset=m,
                            )
                        dst_e = _set_dims(
                            obd2[:],
                            [[OBP, DIM], [1024, 8], [132, 8], [32, 4]],
                            offset=(32 * m) * OBP + m,
                        )
                        if m % 2 == 0:
                            ec = nc.vector.tensor_copy(dst_e, src_e)
                        else:
                            ec = nc.scalar.copy(dst_e, src_e)
                        if t == 0:
                            for ms in (ms2a, ms2b):
                                add_dep_helper(
                                    ec.ins, ms.ins, sync=True,
                                    reason="mask copy after memset",
                                )

                    # ---- b-update: per b4 (4 batches), 8 oc x 2 half matmuls
                    # (512-col halves keep each matmul within one PSUM bank)
                    for b4 in range(8):
                        psb = ps_b.tile([128, 1024], f32, tag="psb")
                        for oc in range(OC):
                            for bh in range(2):
                                nc.tensor.matmul(
                                    psb[:, bh * 512:(bh + 1) * 512],
                                    obd2[:, b4, oc, :],
                                    u_hat[:, 4 * b4 + 2 * bh:4 * b4 + 2 * bh + 2,
                                          oc, :],
                                    start=(oc == 0), stop=(oc == OC - 1),
                                    skip_group_check=True,
                                )
                        # bf16 logits: |b| ~ O(1), bf16 rel err ~0.4% keeps the
                        # routing softmax well inside the 2e-2 tolerance
                        sthi = bu.tile([128, 1024], bf16, tag="sthi")
                        nc.scalar.copy(sthi[:], psb[:])
                        thi = tring.tile([128, 8, 128], bf16, tag="thi")
                        nc.sync.dma_start_transpose(thi[:], sthi[:])
                        # pick diag: blog[p, 4*b4+bq, nh, i] = t[p, bq*2+nh, bq*32+i]
                        thv = _set_dims(
                            thi[:], [[1024, 128], [288, 4], [128, 2], [1, 32]]
                        )
                        nc.vector.tensor_copy(
                            blog[:, 4 * b4:4 * (b4 + 1), :, :], thv
                        )

                        # ---- softmax over i, per 8-batch group
                        if b4 % 2 == 1:
                            g8 = b4 // 2
                            bl = blog[:, 8 * g8:8 * (g8 + 1), :, :]
                            bl3 = bl.rearrange("p b h i -> p (b h) i")
                            mxs = mx[:, 8 * g8:8 * (g8 + 1), :]
                            sms = sm[:, 8 * g8:8 * (g8 + 1), :]
                            smrs = smr[:, 8 * g8:8 * (g8 + 1), :]
                            nc.vector.tensor_reduce(
                                mxs, bl, axis=AX.X, op=OP.max
                            )
                            mxb = _set_dims(
                                mx[:, 0, 0],
                                [[2 * B, 128], [1, 16], [0, NCAP]],
                                offset=16 * g8,
                            )
                            nc.vector.tensor_sub(bl3, bl3, mxb)
                            nc.scalar.activation(bl, bl, AF.Exp)
                            nc.vector.tensor_reduce(
                                sms, bl, axis=AX.X, op=OP.add
                            )
                            nc.vector.reciprocal(smrs, sms)
                            smb = _set_dims(
                                smr[:, 0, 0],
                                [[2 * B, 128], [1, 16], [0, NCAP]],
                                offset=16 * g8,
                            )
                            nc.vector.tensor_mul(
                                c_sb[:, 8 * g8:8 * (g8 + 1), :, :].rearrange(
                                    "p b h i -> p (b h) i"
                                ),
                                bl3, smb,
                            )

                    # ---- o-pass: xbar-transpose u_hat per 2b, matmul with c
                    for b in range(B):
                        cg = b & 3
                        if cg == 0:
                            pso = ps_o.tile([128, 1024], f32, tag="pso")
                        if b % 2 == 0:
                            uht = ring.tile([128, 32, 128], bf16, tag="uht")
                            nc.sync.dma_start_transpose(
                                uht[:],
                                u_hat[:, b:b + 2, :, :].rearrange(
                                    "p b a n -> p (b a n)"
                                ),
                            )
                        b1 = b & 1
                        for nh in range(2):
                            lhs = c_sb[:, b, nh, :]
                            for oh in range(2):
                                rhs = _set_dims(
                                    uht[:],
                                    [[32 * 128, 128], [256, 4], [1, 128]],
                                    offset=(16 * b1 + 8 * oh + nh) * 128,
                                )
                                nc.tensor.matmul(
                                    pso[32 * cg:32 * cg + 32,
                                        oh * 512:(oh + 1) * 512],
                                    lhs, rhs,
                                    start=(nh == 0), stop=(nh == 1),
                                    tile_position=(0, 32 * cg),
                                    skip_group_check=True,
                                )
                        if cg == 3:
                            # 32x32 block transpose; diag becomes stride-33 cols
                            nc.vector.transpose(tr_scr[:], pso[:])
                            for c2 in range(4):
                                bb = b - 3 + c2
                                diag = _set_dims(
                                    tr_scr[:], [[1024, 32], [33, DIM]],
                                    offset=(32 * c2) * 1024,
                                )
                                # o_acc[j, bb, i] overlays onl
                                dst_a = _set_dims(
                                    onl[:], [[O, DIM], [1, NCAP]],
                                    offset=bb * NCAP,
                                )
                                if c2 % 2 == 0:
                                    nc.scalar.copy(dst_a, diag)
                                else:
                                    nc.vector.tensor_copy(dst_a, diag)
                    # o_acc [j, (b,i)] (overlaid on onl) -> transpose on chip
                    # to [i, (b,j)] so the AllReduce buffers stay contiguous
                    # for both the ship and the next round's load
                    nc.vector.transpose(o_t[0:B, :], onl[:])
                    nc.scalar.dma_start(cc_in[t + 1][:], o_t[0:B, :])
                    all_reduce(t + 1)

    nc.compile()
    return nc


def host_prep(u_vecs, W, core):
    ns = slice(core * NL, (core + 1) * NL)
    Wc = np.asarray(W[ns], dtype=np.float32)             # [NL, 16, 1024]
    uc = np.asarray(u_vecs[:, ns, :], dtype=np.float32)  # [B, NL, 16]
    bf = ml_dtypes.bfloat16

    # wl[oc, n8*16+k, g*128+c] = W[g*8+n8, k, oc*128+c]
    wl = (
        Wc.reshape(G, 8, KD, OC, 128)
        .transpose(3, 1, 2, 0, 4)
        .reshape(OC, 128, G * 128)
        .astype(bf)
    )
    tmp = uc.transpose(1, 2, 0).reshape(G, 8, KD, B)     # [g, n8, k, b]
    # ubd[n8*16+k, g*256 + b*8+n8'] = u[b, g*8+n8, k] * (n8 == n8')
    ubd = np.zeros((8, KD, G, B, 8), dtype=np.float32)
    for n8 in range(8):
        ubd[n8, :, :, :, n8] = tmp[:, n8].transpose(1, 0, 2)
    ubd = ubd.reshape(128, G * B * 8).astype(bf)
    # upl[n8*16+k, g*32+b] = u[b, g*8+n8, k]
    upl = tmp.transpose(1, 2, 0, 3).reshape(128, G * B).astype(bf)
    return {"wl": wl, "ubd": ubd, "upl": upl}


def kernel(u_vecs, W):
    from concourse import bass_utils

    if "prog" not in _PROG:
        _PROG["prog"] = build_program(NUM_CORES)
    nc = _PROG["prog"]
    in_maps = [host_prep(u_vecs, W, c) for c in range(NUM_CORES)]
    res = bass_utils.run_bass_kernel_spmd(
        nc, in_maps, core_ids=list(range(NUM_CORES))
    )
    out = np.asarray(res.results[0]["out"], dtype=np.float32)
    return out.reshape(B, NCAP, DIM)
